# revision 6
# baseline (speedup 1.0000x reference)
"""FlowNet-style Correlation (pad=20, max_displacement=20, stride2=2) on 8 TRN2 cores.

Strategy
--------
Data-parallel over batch: core b handles sample b (B=8 == n_cores).

Math: out[b, dy, dx, h, w] = (1/C) * sum_c in1[b,c,h,w] * in2[b,c,h+2dy,w+2dx]
with dy,dx in [-10,10] (441 offsets), zero outside bounds.

Since w and w+2dx share parity, split W into even/odd lanes (parity pi, lane
m = w//2, w = 2m+pi).  For a fixed (h1, dy) and parity, the TensorEngine
computes the all-pairs channel contraction

    P[m, col] = sum_c in1[c, h1, 2m+pi] * in2pad[c, h1+2dy, pi, col]

as a single matmul with K=C=128 on partitions, M=64 lane values (the two
parities stacked on PSUM partition halves), and N=84 (the 64 lanes padded by
10 on each side so shifted columns are always in range; padding is zero so
edge clipping falls out for free).  The needed correlation values are the 21
shifted diagonals  P[m, m + dx + 10]  of each 64x84 rectangle.

Compute engines cannot gather across partitions and DMA cannot reach PSUM, so
the banded rectangles are cast fp32->fp16 (DVE/ACT copies) and DMA'd whole to
HBM; the cheap diagonal extraction happens on the host in numpy.  Rows h2 out
of range are never computed - the output buffer is pre-zeroed, which is
exactly the reference's zero padding.

Inputs are pre-converted to fp16 on the host (PE runs fp16 at 1 col/cycle vs
4x slower for fp32; accumulation stays fp32 in PSUM).
"""

import json

import numpy as np

import concourse.bass as bass
import concourse.mybir as mybir
from concourse.tile import TileContext
from concourse.bass_utils import run_bass_kernel_spmd


# --------------------------------------------------------------------------
# BIR legalizer: the staged walrus rejects instructions with more than one
# embedded semaphore wait ("Too many sync wait commands"), but Tile attaches
# several.  Hoist all-but-one wait onto standalone single-wait EventSemaphore
# instructions on the same engine right before the instruction (the same
# idiom bass's own all-engine barrier uses) — semantics-preserving on
# in-order sequencers.
# --------------------------------------------------------------------------
_MAX_EMBEDDED_WAITS = 1


def _split_sync_waits(bir: bytes):
    j = json.loads(bir)
    n = 0
    for fn in j.get("functions", []):
        for blk in fn.get("blocks", []):
            out = []
            changed = False
            for ins in blk.get("instructions", []):
                si = ins.get("sync_info") or {}
                waits = si.get("on_wait") or []
                if len(waits) > _MAX_EMBEDDED_WAITS:
                    for w in waits[:-_MAX_EMBEDDED_WAITS]:
                        n += 1
                        carrier = {
                            "engine": ins["engine"],
                            "ins": [],
                            "outs": [],
                            "name": f"hw{n}_{ins['name']}",
                            "opcode": "EventSemaphore",
                            "sync_info": {"on_update": [], "on_wait": [w]},
                        }
                        if "debug" in ins:
                            carrier["debug"] = ins["debug"]
                        out.append(carrier)
                    si["on_wait"] = waits[-_MAX_EMBEDDED_WAITS:]
                    ins["sync_info"] = si
                    changed = True
                out.append(ins)
            if changed:
                blk["instructions"] = out
    return (json.dumps(j, separators=(",", ":")).encode(), n) if n else (bir, 0)


_patched = False


def _install_birfix():
    global _patched
    if _patched:
        return
    _patched = True
    import concourse.bass_utils as bu
    import concourse.bass2jax as b2j

    orig = bu.compile_bir_kernel

    def patched(bir_json, tmpdir, neff_name="file.neff"):
        if isinstance(bir_json, str):
            bir_json = bir_json.encode()
        fixed, _ = _split_sync_waits(bir_json)
        return orig(fixed, tmpdir, neff_name)

    bu.compile_bir_kernel = patched
    b2j.compile_bir_kernel = patched


_install_birfix()

B, C, H, W = 8, 128, 96, 128
R = 10                    # displacement radius in stride-2 units
G = 2 * R + 1             # 21 offsets per axis
WP = W // 2               # 64 lanes per parity
PW = R                    # zero padding per side in lane units
WIN = WP + 2 * PW         # 84-wide padded lane row
SLOTS_PER_BANK = 512 // WIN   # 6 fp32 slots of width 84 per 2KB PSUM bank
NBANKS = -(-G // SLOTS_PER_BANK)  # 4 banks hold all 21 slots
OUT_PITCH = G * WIN       # 1764 fp16 per (h1, p) output row


def _valid_dyi(h1):
    """Inclusive range [v0, v1] of dyi = dy + R with 0 <= h1 + 2*dy < H."""
    v0 = max(0, R - h1 // 2)
    v1 = min(G - 1, R + (H - 1 - h1) // 2)
    return v0, v1


def build_program(h_range=None, use_act=False):
    if h_range is None:
        h_range = range(H)
    nc = bass.Bass(
        "TRN2",
        target_bir_lowering=False,
        debug=False,
        enable_asserts=False,
        num_devices=B,
    )
    f16, f32 = mybir.dt.float16, mybir.dt.float32
    a_d = nc.dram_tensor("a", [C, H * W], f16, kind="ExternalInput")
    b_d = nc.dram_tensor("b", [C, H * 2 * WIN], f16, kind="ExternalInput")
    o_d = nc.dram_tensor("o", [H * W, OUT_PITCH], f16, kind="ExternalOutput")

    with TileContext(nc) as tc:
        with tc.tile_pool(name="inp", bufs=1) as pin, \
             tc.tile_pool(name="ps", bufs=2, space="PSUM") as pp, \
             tc.tile_pool(name="st", bufs=3) as pst:
            a_sb = pin.tile([C, H * W], f16)
            b_sb = pin.tile([C, H * 2 * WIN], f16)
            nc.sync.dma_start(out=a_sb[:, :], in_=a_d.ap())
            nc.sync.dma_start(out=b_sb[:, :], in_=b_d.ap())

            for h1 in h_range:
                v0, v1 = _valid_dyi(h1)
                V = v1 - v0 + 1
                ps = pp.tile([C, NBANKS * 512], f32, tag="ps")
                for v in range(V):
                    h2 = h1 + 2 * ((v0 + v) - R)
                    off = 512 * (v // SLOTS_PER_BANK) + WIN * (v % SLOTS_PER_BANK)
                    for pi in range(2):
                        nc.tensor.matmul(
                            ps[pi * WP:(pi + 1) * WP, off:off + WIN],
                            a_sb[:, h1 * W + pi * WP: h1 * W + (pi + 1) * WP],
                            b_sb[:, (h2 * 2 + pi) * WIN: (h2 * 2 + pi + 1) * WIN],
                            start=True, stop=True,
                        )
                st = pst.tile([C, V * WIN], f16, tag="st")
                nb = -(-V // SLOTS_PER_BANK)
                for bk in range(nb):
                    s0 = bk * SLOTS_PER_BANK
                    s1 = min(s0 + SLOTS_PER_BANK, V)
                    src = ps[:, 512 * bk: 512 * bk + (s1 - s0) * WIN]
                    dst = st[:, s0 * WIN: s1 * WIN]
                    if use_act and bk % 2 == 1:
                        nc.scalar.copy(dst, src)
                    else:
                        nc.vector.tensor_copy(out=dst, in_=src)
                nc.sync.dma_start(
                    out=o_d.ap()[h1 * W:(h1 + 1) * W, v0 * WIN:(v0 + V) * WIN],
                    in_=st[:, :],
                )
    return nc


_CACHE = {}


def _get_nc():
    if "nc" not in _CACHE:
        _CACHE["nc"] = build_program()
    return _CACHE["nc"]


def make_in_maps(input1, input2):
    in1 = np.ascontiguousarray(np.asarray(input1, dtype=np.float32))
    in2 = np.ascontiguousarray(np.asarray(input2, dtype=np.float32))
    in_maps = []
    for b in range(B):
        x1 = in1[b].reshape(C, H, WP, 2)          # w = 2m + pi
        a_r = np.ascontiguousarray(x1.transpose(0, 1, 3, 2)).reshape(C, H * W)
        x2 = in2[b].reshape(C, H, WP, 2)
        b_r = np.zeros((C, H, 2, WIN), dtype=np.float32)
        b_r[:, :, 0, PW:PW + WP] = x2[:, :, :, 0]
        b_r[:, :, 1, PW:PW + WP] = x2[:, :, :, 1]
        in_maps.append({
            "a": a_r.astype(np.float16),
            "b": b_r.reshape(C, H * 2 * WIN).astype(np.float16),
        })
    return in_maps


def extract_output(results, h_range=None):
    """results: list (per core) of {"o": np.ndarray} -> [B, 441, H, W] fp32."""
    if h_range is None:
        h_range = range(H)
    p = np.arange(W)
    m_of_p = p % WP
    w_of_p = 2 * (p % WP) + (p // WP)
    inv = np.empty(W, dtype=np.int64)
    inv[w_of_p] = p
    v0s = np.array([_valid_dyi(h)[0] for h in range(H)])
    v1s = np.array([_valid_dyi(h)[1] for h in range(H)])
    dyi = np.arange(G)
    # device writes slot dyi at column offset dyi*WIN (absolute indexing)
    valid = (dyi[None, :] >= v0s[:, None]) & (dyi[None, :] <= v1s[:, None])
    col = m_of_p[:, None] + np.arange(G)[None, :]              # [W, G] in [0, 84)

    out = np.zeros((B, G * G, H, W), dtype=np.float32)
    for b in range(B):
        st = results[b]["o"].astype(np.float32).reshape(H, W, G, WIN)
        u = np.take_along_axis(st, col[None, :, None, :], axis=3)    # [H,W,Gdy,Gdx]
        u = np.where(valid[:, None, :, None], u, np.float32(0.0))
        u *= np.float32(1.0 / C)
        v = u.transpose(2, 3, 0, 1).reshape(G * G, H, W)
        out[b] = v[:, :, inv]
    if len(h_range) != H:
        mask = np.zeros(H, dtype=bool)
        mask[list(h_range)] = True
        out[:, :, ~mask, :] = 0.0
    return out


def run_device(nc, in_maps, trace=False, **kwargs):
    return run_bass_kernel_spmd(nc, in_maps, core_ids=list(range(len(in_maps))),
                                trace=trace, **kwargs)


def kernel(input1, input2):
    nc = _get_nc()
    in_maps = make_in_maps(input1, input2)
    res = run_device(nc, in_maps)
    return extract_output(res.results)
